# revision 2
# baseline (speedup 1.0000x reference)
"""Deep 6-layer GCN autoencoder on 8 Trainium2 NeuronCores.

Strategy (row-sharded):
- Nodes row-sharded 8 ways. Each layer: xw = h_local @ W computed per core,
  AllGather -> full fp16 table [N, 128] in DRAM (32 features used, 256B row
  stride as required by dma_gather).
- SpMM A @ xw: per 128-row output tile, host-deduped column list is gathered
  from the table with 64B-elem dma_gather (4 SWDGE queues), then a fp16
  "scatter matrix" S (slots x rows, values = edge vals summed per (col,row))
  contracts on the TensorEngine with 4-way column tiling, accumulating in PSUM.
- Decoder: z^T feature-major AllGather -> z_all [16, N] fp16 in SBUF;
  out rows = z_local_tile^T.T @ z_all chunks (fp16 matmuls, fp32 PSUM),
  written as [NPC, N] fp32 per core.
"""
import numpy as np

try:
    import concourse.bass as bass
except ImportError:
    import sys
    sys.path.insert(0, "/opt/trn_rl_repo")
    import concourse.bass as bass
import concourse.bacc as bacc
import concourse.mybir as mybir
import concourse.tile as tile
from concourse import bass_utils, ap_utils
from concourse.masks import make_identity

C = 8  # cores
VARIANT = "full"  # ablation hook used by test_ablate.py


def dma_gather_raw(eng, out_ap, in_ap, idxs_ap, num_idxs, elem_size, elem_step,
                   queue_num=0):
    """bass.dma_gather without the elem%256B restriction (valid for
    non-transpose mode; only the table row stride must be 256B-aligned)."""
    assert idxs_ap.dtype == mybir.dt.int16
    assert in_ap.dtype == out_ap.dtype
    assert in_ap.space == bass.MemorySpace.DRAM
    assert idxs_ap.space == bass.MemorySpace.SBUF
    assert out_ap.space == bass.MemorySpace.SBUF
    assert ap_utils.ap_is_contiguous(out_ap.ap[1:])
    assert ap_utils.ap_is_contiguous(idxs_ap.ap[1:])
    assert in_ap.ap[-1][1] == out_ap.ap[-1][1] == elem_size
    assert out_ap.ap[0][1] * out_ap.ap[1][1] == ((num_idxs + 127) // 128) * 128
    assert in_ap.ap[0][0] == elem_step
    stride_bytes = elem_step * mybir.dt.size(in_ap.dtype)
    assert stride_bytes % 256 == 0 and stride_bytes // 256 < 256
    _in_ap = eng.lower_ap_dma(in_ap, for_custom_bir_dma=True)
    _idxs_ap = eng.lower_ap(idxs_ap)
    _out_ap = eng.lower_ap(out_ap)
    return eng.add_instruction(
        mybir.InstDMAGatherAnt(
            name=eng.bass.get_next_instruction_name(),
            ins=[*_in_ap, _idxs_ap, eng.lower_val_access(eng.to_reg(num_idxs))],
            outs=[_out_ap],
            transpose=False,
            num_idxs=num_idxs,
            elem_size=elem_size,
            stride_bytes_256=stride_bytes // 256,
            gen_mode=0,
            single_packet=True,
            queue_num=queue_num,
            sbuf_tokens_per_rank=0,
            sbuf_free_dim_per_rank=0,
            sbuf_free_dim_pad_per_rank=0,
            sbuf_byte_offset=0,
        ))


def _wrap_idx(idx_list):
    """dma_gather wrapped int16 layout: idx n -> partition n%16 (replicated
    across all eight 16-partition bands), col n//16."""
    n = len(idx_list)
    cols = n // 16
    band = np.asarray(idx_list, dtype=np.int16).reshape(cols, 16).T  # [16, cols]
    return np.tile(band, (8, 1))  # [128, cols]


def _preprocess(adj_row, adj_col, adj_vals, N, NPC, NT):
    """Per-core, per-128-row-tile deduped gather lists + fp16 scatter matrices."""
    rows = np.asarray(adj_row).astype(np.int64)
    cols = np.asarray(adj_col).astype(np.int64)
    vals = np.asarray(adj_vals).astype(np.float32)

    order = np.argsort(rows, kind="stable")
    rows, cols, vals = rows[order], cols[order], vals[order]
    tile_los = [c * NPC + t * 128 for c in range(C) for t in range(NT)]
    bounds = np.searchsorted(rows, np.asarray(tile_los + [C * NPC]))

    per_core = []
    tiles = []  # (c, t, dcols, inv, r_loc, v)
    nmax = 0
    for c in range(C):
        for t in range(NT):
            lo = c * NPC + t * 128
            hi = min(c * NPC + (t + 1) * 128, (c + 1) * NPC)
            gi = c * NT + t
            a, b = bounds[gi], bounds[gi + 1]
            r_loc = rows[a:b] - lo
            dcols, inv = np.unique(cols[a:b], return_inverse=True)
            nmax = max(nmax, len(dcols))
            tiles.append((c, t, dcols, inv, r_loc, vals[a:b]))

    NB = max((nmax + 127) // 128, 1)
    NSLOT = NB * 128
    for c in range(C):
        S_core = np.zeros((NT, NB, 128, 128), np.float32)
        idx_core = np.zeros((NT, 128, NSLOT // 16), np.int16)
        for (cc, t, dcols, inv, r_loc, v) in tiles:
            if cc != c:
                continue
            idx_list = np.zeros(NSLOT, np.int16)
            idx_list[:len(dcols)] = dcols
            idx_core[t] = _wrap_idx(idx_list)
            np.add.at(S_core[t], (inv // 128, inv % 128, r_loc), v)
        # SBUF layout: [128 slots, NT*NB*128] fp16 (block b=(t*NB+j) at cols b*128..)
        s_host = S_core.reshape(NT * NB, 128, 128).transpose(1, 0, 2) \
                       .reshape(128, NT * NB * 128).astype(np.float16)
        per_core.append((s_host, idx_core))
    return per_core, NB


def _build(N, IN_DIM, NPC, NT, NB, unscale):
    """Build the SPMD bass program (identical for all cores)."""
    D = 32
    NSLOT = NB * 128
    NCH = 1                  # decoder column chunks: smallest with width <= 512
    while N % NCH or N // NCH > 512:
        NCH += 1
    CHW = N // NCH
    KCH = IN_DIM // 128      # phase-A contraction chunks

    nc = bacc.Bacc("TRN2", target_bir_lowering=False, debug=False,
                   num_devices=C, num_swdge_queues=4)
    x_in = nc.dram_tensor("x_in", [NPC, IN_DIM], mybir.dt.float32, kind="ExternalInput")
    s_in = nc.dram_tensor("s_in", [128, NT * NB * 128], mybir.dt.float16, kind="ExternalInput")
    idx_in = nc.dram_tensor("idx_in", [NT, 128, NSLOT // 16], mybir.dt.int16, kind="ExternalInput")
    w0_in = nc.dram_tensor("w0_in", [128, KCH * D], mybir.dt.float32, kind="ExternalInput")
    wl_in = nc.dram_tensor("wl_in", [6 * D, D], mybir.dt.float32, kind="ExternalInput")
    out_d = nc.dram_tensor("out", [NPC, N], mybir.dt.float32, kind="ExternalOutput")

    # gather split: chunks of <=8 blocks (<=1024 idxs, the SWDGE ring capacity)
    splits = []
    b0 = 0
    while b0 < NB:
        nb = min(8, NB - b0)
        splits.append((b0, nb))
        b0 += nb

    with tile.TileContext(nc) as tc:
        with tc.tile_pool(name="res", bufs=1) as res, \
             tc.tile_pool(name="gp", bufs=3) as gp, \
             tc.tile_pool(name="work", bufs=2) as work, \
             tc.tile_pool(name="dstage", bufs=4) as dstage, \
             tc.tile_pool(name="pxw", bufs=2, space="PSUM") as pxw, \
             tc.tile_pool(name="psc", bufs=2, space="PSUM") as psc, \
             tc.tile_pool(name="pbig", bufs=3, space="PSUM") as pbig, \
             tc.tile_pool(name="dram", bufs=1, space="DRAM") as dram:

            # resident tensors
            s_sb = res.tile([128, NT * NB * 128], mybir.dt.float16)
            nc.sync.dma_start(s_sb[:], s_in[:, :])
            idx_sb = res.tile([128, NT * NSLOT // 16], mybir.dt.int16)
            for t in range(NT):
                nc.sync.dma_start(
                    idx_sb[:, t * (NSLOT // 16):(t + 1) * (NSLOT // 16)],
                    idx_in[t, :, :])
            w0_sb = res.tile([128, KCH * D], mybir.dt.float32)
            nc.sync.dma_start(w0_sb[:], w0_in[:, :])
            wl_sb = res.tile([D, 6 * D], mybir.dt.float32)
            for l in range(6):
                nc.sync.dma_start(wl_sb[:, l * D:(l + 1) * D],
                                  wl_in[l * D:(l + 1) * D, :])
            ident = res.tile([128, 128], mybir.dt.float32)
            make_identity(nc, ident[:])
            ht_a = res.tile([D, NT * 128], mybir.dt.float32)
            ht_b = res.tile([D, NT * 128], mybir.dt.float32)
            zt_sb = res.tile([16, NT * 128], mybir.dt.float16)
            z_all = res.tile([16, C * NPC], mybir.dt.float16)

            shard = dram.tile([NPC, 128], mybir.dt.float16)
            table = dram.tile([N, 128], mybir.dt.float16)
            zeros128 = res.tile([128, 128], mybir.dt.float16)
            nc.vector.memset(zeros128[:], 0.0)
            for t in range(NT):
                R = min(128, NPC - t * 128)
                nc.sync.dma_start(shard[t * 128:t * 128 + R, :], zeros128[0:R, :])
            z_shard = dram.tile([16, NPC], mybir.dt.float16)
            z_stack = dram.tile([C * 16, NPC], mybir.dt.float16)

            def stage_xw(t, psum_xw):
                """convert xw psum [128, 32] fp32 -> fp16 -> shard rows."""
                R = min(128, NPC - t * 128)
                st = work.tile([128, D], mybir.dt.float16, name="xst", tag="xst")
                nc.scalar.activation(out=st[:], in_=psum_xw[:],
                                     func=mybir.ActivationFunctionType.Copy)
                nc.sync.dma_start(shard[t * 128:t * 128 + R, 0:D], st[0:R, :])

            # ---- phase A: xw0 = X @ W0 ----
            for t in range(NT):
                R = min(128, NPC - t * 128)
                xt = work.tile([128, IN_DIM], mybir.dt.float32, name="xt", tag="xt")
                if R < 128:
                    nc.vector.memset(xt[:], 0.0)
                nc.sync.dma_start(xt[0:R, :], x_in[t * 128:t * 128 + R, :])
                psum_xw = pxw.tile([128, D], mybir.dt.float32, name="pxw_a", tag="pxw")
                for k in range(KCH):
                    ptr = pbig.tile([128, 128], mybir.dt.float32, name="ptr", tag="pbig")
                    nc.tensor.transpose(out=ptr[:], in_=xt[:, k * 128:(k + 1) * 128],
                                        identity=ident[:])
                    xtT = work.tile([128, 128], mybir.dt.float32, name="xtT", tag="xtT")
                    nc.vector.tensor_copy(out=xtT[:], in_=ptr[:])
                    nc.tensor.matmul(out=psum_xw[:], lhsT=xtT[:],
                                     rhs=w0_sb[:, k * D:(k + 1) * D],
                                     start=(k == 0), stop=(k == KCH - 1))
                stage_xw(t, psum_xw)

            # ---- layers ----
            ht_cur, ht_nxt = ht_a, ht_b
            for l in range(7):
                nc.gpsimd.collective_compute(
                    "AllGather", mybir.AluOpType.bypass,
                    replica_groups=[list(range(C))],
                    ins=[shard[:, :]], outs=[table[:, :]])
                g0 = None
                for t in range(NT):
                    if VARIANT == "gather1" and t > 0:
                        g = g0
                    else:
                        g = gp.tile([128, NB * D], mybir.dt.float16, name=f"g{l}_{t}", tag="g")
                        for q, (b0, nb) in enumerate(splits):
                            ni = nb * 128
                            dma_gather_raw(
                                nc.gpsimd,
                                out_ap=g[:, b0 * D:(b0 + nb) * D].rearrange(
                                    "p (m d) -> p m d", d=D),
                                in_ap=table[:, :D],
                                idxs_ap=idx_sb[:, (t * NSLOT + b0 * 128) // 16:
                                                  (t * NSLOT + (b0 + nb) * 128) // 16],
                                num_idxs=ni, elem_size=D, elem_step=128,
                                queue_num=q % 4)
                        g0 = g
                    psum = psc.tile([128, 128], mybir.dt.float32, name="psc", tag="psc")
                    nblk_eff = 4 if VARIANT == "scatter1" else NB
                    ngrp = [0, 0, 0, 0]
                    for j in range(nblk_eff):
                        ngrp[j % 4] += 1
                    cnt = [0, 0, 0, 0]
                    for j in range(nblk_eff):
                        cg = j % 4
                        cnt[cg] += 1
                        nc.tensor.matmul(
                            out=psum[32 * cg:32 * cg + 32, :],
                            lhsT=g[:, j * D:(j + 1) * D],
                            rhs=s_sb[:, (t * NB + j) * 128:(t * NB + j + 1) * 128],
                            start=(cnt[cg] == 1), stop=(cnt[cg] == ngrp[cg]),
                            tile_position=(0, 32 * cg), skip_group_check=True)
                    red = work.tile([32, 128], mybir.dt.float32, name="red", tag="red")
                    nc.vector.tensor_copy(out=red[:], in_=psum[0:32, :])
                    for cg in range(1, 4):
                        if ngrp[cg]:
                            nc.vector.tensor_add(out=red[:], in0=red[:],
                                                 in1=psum[32 * cg:32 * cg + 32, :])
                    if l < 6:
                        nc.scalar.activation(
                            out=ht_nxt[:, t * 128:(t + 1) * 128], in_=red[:],
                            func=mybir.ActivationFunctionType.Relu)
                        psum_xw = pxw.tile([128, D], mybir.dt.float32,
                                           name="pxw_l", tag="pxw")
                        nc.tensor.matmul(
                            out=psum_xw[:],
                            lhsT=ht_nxt[:, t * 128:(t + 1) * 128],
                            rhs=wl_sb[:, l * D:(l + 1) * D],
                            start=True, stop=True)
                        stage_xw(t, psum_xw)
                    else:
                        nc.scalar.activation(
                            out=zt_sb[0:16, t * 128:(t + 1) * 128], in_=red[0:16, :],
                            func=mybir.ActivationFunctionType.Copy)
                ht_cur, ht_nxt = ht_nxt, ht_cur

            # ---- z AllGather (feature-major) ----
            nc.sync.dma_start(z_shard[:, :], zt_sb[0:16, 0:NPC])
            nc.gpsimd.collective_compute(
                "AllGather", mybir.AluOpType.bypass,
                replica_groups=[list(range(C))],
                ins=[z_shard[:, :]], outs=[z_stack[:, :]])
            for d in range(C):
                nc.sync.dma_start(z_all[0:16, d * NPC:(d + 1) * NPC],
                                  z_stack[d * 16:(d + 1) * 16, :])

            # ---- decoder ----
            if VARIANT != "nodec":
                for t in range(NT):
                    R = min(128, NPC - t * 128)
                    for ch in range(NCH):
                        pd = pbig.tile([128, CHW], mybir.dt.float32, name="pd", tag="pbig")
                        nc.tensor.matmul(
                            out=pd[:],
                            lhsT=zt_sb[0:16, t * 128:(t + 1) * 128],
                            rhs=z_all[0:16, ch * CHW:(ch + 1) * CHW],
                            start=True, stop=True)
                        st = dstage.tile([128, CHW], mybir.dt.float32, name="dst", tag="dst")
                        if ch % 2 == 0:
                            nc.vector.tensor_scalar_mul(out=st[:], in0=pd[:],
                                                        scalar1=float(unscale))
                        else:
                            nc.scalar.activation(out=st[:], in_=pd[:],
                                                 func=mybir.ActivationFunctionType.Copy,
                                                 scale=float(unscale))
                        if VARIANT == "nodecdma" and not (t == 0 and ch == 0):
                            continue
                        nc.sync.dma_start(
                            out_d[t * 128:t * 128 + R, ch * CHW:(ch + 1) * CHW],
                            st[0:R, :])
    nc.compile()
    return nc


_CACHE = {}


def _get_program(N, IN_DIM, NPC, NT, NB, unscale):
    key = (N, IN_DIM, NPC, NT, NB, float(unscale))
    if key not in _CACHE:
        _CACHE[key] = _build(N, IN_DIM, NPC, NT, NB, unscale)
    return _CACHE[key]


def _calibrate_scales(features, adj_row, adj_col, adj_vals, Ws):
    """Cheap fp32 host forward pass to pick per-layer normalizers alpha_l so
    the fp16 activation tables stay near max-abs 64 (relu is positively
    homogeneous, so scaling commutes; undone once in the decoder)."""
    N = features.shape[0]
    rows = np.asarray(adj_row).astype(np.int64)
    cols = np.asarray(adj_col).astype(np.int64)
    vals = np.asarray(adj_vals).astype(np.float32)

    def spmm(x):
        out = np.zeros((N, x.shape[1]), np.float32)
        np.add.at(out, rows, vals[:, None] * x[cols])
        return out

    alphas = []
    h = np.asarray(features, np.float32)
    for l in range(7):
        u = h @ np.asarray(Ws[l], np.float32)
        m = float(np.abs(u).max()) or 1.0
        alphas.append(64.0 / m)
        a = spmm(u)
        h = np.maximum(a, 0) if l < 6 else a
    return alphas


def _make_in_maps(features, adj_row, adj_col, adj_vals, Ws):
    N, IN_DIM = features.shape
    NPC = N // C
    NT = (NPC + 127) // 128
    per_core, NB = _preprocess(adj_row, adj_col, adj_vals, N, NPC, NT)
    alphas = _calibrate_scales(features, adj_row, adj_col, adj_vals, Ws)

    D = 32
    KCH = IN_DIM // 128
    # device weight l is W_l * alpha_l / alpha_{l-1} (alpha_{-1}=1); the
    # decoder output is then scaled by alpha_6^2, undone by `unscale`.
    W0 = np.asarray(Ws[0], np.float32) * alphas[0]
    w0_host = W0.reshape(KCH, 128, D).transpose(1, 0, 2).reshape(128, KCH * D)
    wl_host = np.zeros((6, D, D), np.float32)
    for i in range(1, 7):
        w = np.asarray(Ws[i], np.float32) * (alphas[i] / alphas[i - 1])
        wl_host[i - 1, :, :w.shape[1]] = w
    wl_host = wl_host.reshape(6 * D, D)
    unscale = 1.0 / (alphas[6] ** 2)

    feats = np.asarray(features, np.float32)
    in_maps = []
    for c in range(C):
        s_host, idx_core = per_core[c]
        in_maps.append({
            "x_in": np.ascontiguousarray(feats[c * NPC:(c + 1) * NPC]),
            "s_in": s_host,
            "idx_in": idx_core,
            "w0_in": w0_host,
            "wl_in": wl_host,
        })
    return in_maps, (N, IN_DIM, NPC, NT, NB, unscale)


def _postprocess(out_stack, dims):
    """out_stack: [C, NPC, N] device output -> full [N, N] fp32."""
    return np.asarray(out_stack, np.float32).reshape(dims[0], dims[0])


def kernel(features, adj_row, adj_col, adj_vals, W0, W1, W2, W3, W4, W5, W6):
    in_maps, dims = _make_in_maps(features, adj_row, adj_col, adj_vals,
                                  [W0, W1, W2, W3, W4, W5, W6])
    nc = _get_program(*dims)
    res = bass_utils.run_bass_kernel_spmd(nc, in_maps, core_ids=list(range(C)))
    full = _postprocess(np.stack([res.results[c]["out"] for c in range(C)]), dims)
    return full.reshape(-1)



# revision 26
# speedup vs baseline: 3.3851x; 3.3851x over previous
"""Deep 6-layer GCN autoencoder on 8 Trainium2 NeuronCores.

Strategy (row-sharded, v2):
- Nodes row-sharded 8 ways (NPC=1250/core, padded to 1280 = NT*128).
- Per layer: xw = h_local @ W staged by the scalar engine directly into a
  row-interleaved SBUF shard [128, NT*32] fp16 (partition = local_row%128,
  col block = local_row//128).  One DMA writes it densely to DRAM
  [1280, 32]; an AllGather concatenates all cores -> table [10240, 32]
  fp16 in DRAM (viewed as 2560 lines x 256B; 4 node-rows per line).
- SpMM A @ xw: per 128-row output tile, host-deduped column lists are
  split by col%4 residue classes; each class is one dma_gather with
  elem_size=32 (64B) from the 256B-strided line view at byte offset
  64*r, so the dense table needs no 256B row padding.  A fp16 scatter
  matrix S (slots x rows, edge vals summed per (col,row)) contracts on
  the TensorEngine with 4-way column-group tiling, accumulating in PSUM.
- Decoder: z^T feature-major AllGather -> z_all [16, N] fp16 in SBUF;
  out rows = z_local_tile^T.T @ z_all chunks; psum chunks are scaled to
  fit fp16 and written as [NPC, N] fp16 per core (host de-scales to f32).
"""
import numpy as np

try:
    import concourse.bass as bass
except ImportError:
    import sys
    sys.path.insert(0, "/opt/trn_rl_repo")
    import concourse.bass as bass
import concourse.bacc as bacc
import concourse.mybir as mybir
import concourse.tile as tile
from concourse import bass_utils, ap_utils
from concourse.masks import make_identity

C = 8  # cores
VARIANT = "full"  # ablation hook


def dma_gather_raw(eng, out_ap, in_ap, idxs_ap, num_idxs, elem_size, elem_step,
                   queue_num=0):
    """bass.dma_gather without the elem%256B restriction (valid for
    non-transpose mode; only the table row stride must be 256B-aligned)."""
    assert idxs_ap.dtype == mybir.dt.int16
    assert in_ap.dtype == out_ap.dtype
    assert in_ap.space == bass.MemorySpace.DRAM
    assert idxs_ap.space == bass.MemorySpace.SBUF
    assert out_ap.space == bass.MemorySpace.SBUF
    assert ap_utils.ap_is_contiguous(out_ap.ap[1:])
    assert ap_utils.ap_is_contiguous(idxs_ap.ap[1:])
    assert in_ap.ap[-1][1] == out_ap.ap[-1][1] == elem_size
    assert out_ap.ap[0][1] * out_ap.ap[1][1] == ((num_idxs + 127) // 128) * 128
    assert in_ap.ap[0][0] == elem_step
    stride_bytes = elem_step * mybir.dt.size(in_ap.dtype)
    assert stride_bytes % 256 == 0 and stride_bytes // 256 < 256
    _in_ap = eng.lower_ap_dma(in_ap, for_custom_bir_dma=True)
    _idxs_ap = eng.lower_ap(idxs_ap)
    _out_ap = eng.lower_ap(out_ap)
    return eng.add_instruction(
        mybir.InstDMAGatherAnt(
            name=eng.bass.get_next_instruction_name(),
            ins=[*_in_ap, _idxs_ap, eng.lower_val_access(eng.to_reg(num_idxs))],
            outs=[_out_ap],
            transpose=False,
            num_idxs=num_idxs,
            elem_size=elem_size,
            stride_bytes_256=stride_bytes // 256,
            gen_mode=0,
            single_packet=True,
            queue_num=queue_num,
            sbuf_tokens_per_rank=0,
            sbuf_free_dim_per_rank=0,
            sbuf_free_dim_pad_per_rank=0,
            sbuf_byte_offset=0,
        ))


def _wrap_idx(idx_list):
    """dma_gather wrapped int16 layout: idx n -> partition n%16 (replicated
    across all eight 16-partition bands), col n//16."""
    n = len(idx_list)
    cols = n // 16
    band = np.asarray(idx_list, dtype=np.int16).reshape(cols, 16).T  # [16, cols]
    return np.tile(band, (8, 1))  # [128, cols]


def _preprocess(adj_row, adj_col, adj_vals, N, NPC, NT, reg_of):
    """Per-core, per-128-row-tile, per-residue deduped gather lists + fp16
    scatter matrices.  reg_of[d][o] = exchange-table region on core d that
    holds owner o's shard (measured at runtime; the driver's logical->
    physical NC remap makes this differ from d^o)."""
    rows = np.asarray(adj_row).astype(np.int64)
    cols = np.asarray(adj_col).astype(np.int64)
    vals = np.asarray(adj_vals).astype(np.float32)
    LPC = NT * 128 // 4      # lines per core block

    order = np.argsort(rows, kind="stable")
    rows, cols, vals = rows[order], cols[order], vals[order]
    tile_los = [c * NPC + t * 128 for c in range(C) for t in range(NT)]
    bounds = np.searchsorted(rows, np.asarray(tile_los + [C * NPC]))

    tiles = []  # (c, t, per-residue (lines, inv_slot_in_group), r_loc, v, grp_of_edge)
    cnt_max = [0, 0, 0, 0]
    for c in range(C):
        for t in range(NT):
            lo = c * NPC + t * 128
            gi = c * NT + t
            a, b = bounds[gi], bounds[gi + 1]
            r_loc = rows[a:b] - lo
            e_cols = cols[a:b]
            # table DRAM order (q, reg, tsrc, d): row q*C*NT + reg*NT + tsrc,
            # line = row//4, residue = row%4 = (reg*NT + tsrc) % 4  (C*NT%4==0)
            e_reg = reg_of[c][e_cols // NPC]
            e_q = (e_cols % NPC) % 128
            e_ts = (e_cols % NPC) // 128
            e_row = e_q * (C * NT) + e_reg * NT + e_ts
            e_res = e_row % 4
            groups = []
            for r in range(4):
                m = e_res == r
                glines = e_row[m] // 4
                dlines, inv = np.unique(glines, return_inverse=True)
                cnt_max[r] = max(cnt_max[r], len(dlines))
                groups.append((dlines, inv, m))
            tiles.append((c, t, groups, r_loc, vals[a:b]))

    NBr = [max((m + 127) // 128, 1) for m in cnt_max]
    offB = np.concatenate([[0], np.cumsum(NBr)])
    NB = int(offB[-1])
    NSLOT = NB * 128
    per_core = []
    for c in range(C):
        S_core = np.zeros((NT, NB, 128, 128), np.float32)
        idx_core = np.zeros((NT, 128, NSLOT // 16), np.int16)
        for (cc, t, groups, r_loc, v) in tiles:
            if cc != c:
                continue
            idx_list = np.zeros(NSLOT, np.int16)
            for r in range(4):
                dlines, inv, m = groups[r]
                s0 = int(offB[r]) * 128
                idx_list[s0:s0 + len(dlines)] = dlines
                slot = s0 + inv
                np.add.at(S_core[t], (slot // 128, slot % 128, r_loc[m]), v[m])
            idx_core[t] = _wrap_idx(idx_list)
        # SBUF layout: [128 slots, NT*NB*128] fp16 (block b=(t*NB+j) at cols b*128..)
        s_host = S_core.reshape(NT * NB, 128, 128).transpose(1, 0, 2) \
                       .reshape(128, NT * NB * 128).astype(np.float16)
        per_core.append((s_host, idx_core))
    return per_core, NBr


def _build_probe():
    """Tiny SPMD program measuring which rank's data lands in each exchange
    region (the driver's logical->physical NC remap breaks the naive d^r
    rule).  Same broadcast slots/sem protocol as the main kernel."""
    RD = 64
    nc = bacc.Bacc("TRN2", target_bir_lowering=False, debug=False,
                   num_devices=C, num_swdge_queues=4)
    src_in = nc.dram_tensor("src_in", [128, RD], mybir.dt.int16, kind="ExternalInput")
    out_d = nc.dram_tensor("out", [128, C * RD], mybir.dt.int16, kind="ExternalOutput")
    with tile.TileContext(nc) as tc:
        with tc.tile_pool(name="res", bufs=1) as res, \
             tc.tile_pool(name="dram", bufs=1, space="DRAM") as dram:
            data_sem = nc.alloc_semaphore("data_sem")
            lsem = nc.alloc_semaphore("lsem")
            flush_sem = nc.alloc_semaphore("flush_sem")
            with tc.tile_critical(sync_engine=mybir.EngineType.Pool, name="clr"):
                nc.gpsimd.sem_clear(data_sem)
                nc.gpsimd.sem_clear(lsem)
                nc.gpsimd.sem_clear(flush_sem)
            bar_i = dram.tile([1, 4], mybir.dt.int32)
            bar_o = dram.tile([C, 4], mybir.dt.int32)
            nc.gpsimd.collective_compute(
                "AllGather", mybir.AluOpType.bypass,
                replica_groups=[list(range(C))],
                ins=[bar_i[:, :]], outs=[bar_o[:, :]])
            tb = res.tile([128, C * RD], mybir.dt.int16)
            nc.vector.memset(tb[:], 0.0)
            src = res.tile([128, RD], mybir.dt.int16)
            nc.sync.dma_start(src[:], src_in[:, :])
            nc.vector.tensor_copy(out=tb[:, 0:RD], in_=src[:])
            for r in range(1, C):
                rdests = [None] * C
                rdests[r] = (0, r)
                nc.gpsimd.remote_dma_broadcast(
                    out_ap=tb[:, r * RD:(r + 1) * RD],
                    in_ap=tb[:, 0:RD],
                    remote_sem=data_sem, local_sem=lsem,
                    rdests=rdests, queue_num=0)
            nc.gpsimd.trigger_dma(count=None, queue_num=0)
            with tc.tile_critical(sync_engine=mybir.EngineType.SP, name="xchg"):
                nc.sync.wait_ge(data_sem, 14)
                nc.sync.dma_start(out_d[:, :], tb[:]).then_inc(flush_sem, 16)
                nc.sync.wait_ge(flush_sem, 16)
    nc.compile()
    return nc


_REG_OF = None


def _get_reg_of():
    """reg_of[d][o] = region index on core d holding owner o's shard."""
    global _REG_OF
    if _REG_OF is not None:
        return _REG_OF
    RD = 64
    nc = _build_probe()
    pats, in_maps = [], []
    for c in range(C):
        flat = (np.arange(128 * RD) % 4096).reshape(128, RD)
        pat = (c * 4096 + flat).astype(np.int16)
        pats.append(pat)
        in_maps.append({"src_in": pat})
    res = bass_utils.run_bass_kernel_spmd(nc, in_maps, core_ids=list(range(C)))
    reg_of = np.full((C, C), -1, np.int64)
    for d in range(C):
        out = res.results[d]["out"]
        for r in range(C):
            reg = out[:, r * RD:(r + 1) * RD]
            hits = [c for c in range(C) if np.array_equal(reg, pats[c])]
            assert len(hits) == 1, f"probe: core {d} region {r} ambiguous {hits}"
            reg_of[d][hits[0]] = r
    assert (reg_of >= 0).all()
    _REG_OF = reg_of
    return reg_of


def _build(N, IN_DIM, NPC, NT, NBr, out_scale):
    """Build the SPMD bass program (identical for all cores)."""
    D = 32
    NBr = list(NBr)
    offB = [0]
    for x in NBr:
        offB.append(offB[-1] + x)
    NB = offB[-1]
    NSLOT = NB * 128
    NPCP = NT * 128          # padded rows per core
    LPC = NPCP // 4          # table lines per core block
    NLINE = C * LPC
    NCH = 1                  # decoder column chunks: width <= 500 (one psum bank)
    while N % NCH or N // NCH > 500:
        NCH += 1
    CHW = N // NCH
    DST = 4                  # psum chunks per fp16 staging tile
    KCH = IN_DIM // 128      # phase-A contraction chunks

    nc = bacc.Bacc("TRN2", target_bir_lowering=False, debug=False,
                   num_devices=C, num_swdge_queues=4)
    x_in = nc.dram_tensor("x_in", [NPC, IN_DIM], mybir.dt.float32, kind="ExternalInput")
    s_in = nc.dram_tensor("s_in", [128, NT * NB * 128], mybir.dt.float16, kind="ExternalInput")
    idx_in = nc.dram_tensor("idx_in", [NT, 128, NSLOT // 16], mybir.dt.int16, kind="ExternalInput")
    w0_in = nc.dram_tensor("w0_in", [128, KCH * D], mybir.dt.float32, kind="ExternalInput")
    wl_in = nc.dram_tensor("wl_in", [6 * D, D], mybir.dt.float32, kind="ExternalInput")
    out_d = nc.dram_tensor("out", [NPC, N], mybir.dt.float16, kind="ExternalOutput")

    # per-residue gather splits: chunks of <=8 blocks (<=1024 idxs per gather)
    splits = []  # (residue, slot block b0, nblocks)
    for r in range(4):
        b0 = 0
        while b0 < NBr[r]:
            nb = min(8, NBr[r] - b0)
            splits.append((r, offB[r] + b0, nb))
            b0 += nb

    with tile.TileContext(nc) as tc:
        with tc.tile_pool(name="res", bufs=1) as res, \
             tc.tile_pool(name="gp", bufs=3) as gp, \
             tc.tile_pool(name="work", bufs=2) as work, \
             tc.tile_pool(name="dstage", bufs=3) as dstage, \
             tc.tile_pool(name="pxw", bufs=2, space="PSUM") as pxw, \
             tc.tile_pool(name="psc", bufs=2, space="PSUM") as psc, \
             tc.tile_pool(name="pbig", bufs=3, space="PSUM") as pbig, \
             tc.tile_pool(name="dram", bufs=1, space="DRAM") as dram:

            # resident tensors
            s_sb = res.tile([128, NT * NB * 128], mybir.dt.float16)
            nc.sync.dma_start(s_sb[:], s_in[:, :])
            idx_sb = res.tile([128, NT * NSLOT // 16], mybir.dt.int16)
            for t in range(NT):
                nc.sync.dma_start(
                    idx_sb[:, t * (NSLOT // 16):(t + 1) * (NSLOT // 16)],
                    idx_in[t, :, :])
            w0_sb = res.tile([128, KCH * D], mybir.dt.float32)
            nc.sync.dma_start(w0_sb[:], w0_in[:, :])
            wl_sb = res.tile([D, 6 * D], mybir.dt.float32)
            for l in range(6):
                nc.sync.dma_start(wl_sb[:, l * D:(l + 1) * D],
                                  wl_in[l * D:(l + 1) * D, :])
            ident = res.tile([128, 128], mybir.dt.float32)
            make_identity(nc, ident[:])
            ht_a = res.tile([D, NT * 128], mybir.dt.float32)
            ht_b = res.tile([D, NT * 128], mybir.dt.float32)
            zt_sb = res.tile([16, NT * 128], mybir.dt.float16)
            z_all = res.tile([16, C * NPC], mybir.dt.float16)
            # double-buffered exchange table: region r holds shard of core
            # (me ^ r); region 0 is the locally-staged shard.
            tb = [res.tile([128, C * NT * D], mybir.dt.float16, name=f"tb{p}")
                  for p in range(2)]

            table_d = [dram.tile([C * NPCP, D], mybir.dt.float16, name=f"table_d{p}")
                       for p in range(2)]
            z_shard = dram.tile([16, NPC], mybir.dt.float16)
            z_stack = dram.tile([C * 16, NPC], mybir.dt.float16)

            data_sem = [nc.alloc_semaphore(f"data_sem{p}") for p in range(2)]
            lsem = [nc.alloc_semaphore(f"lsem{p}") for p in range(2)]
            flush_sem = nc.alloc_semaphore("flush_sem")
            RD = NT * D  # region width (cols) in the exchange table

            # NRT does not zero semaphores between model loads: clear ours,
            # then barrier so no core's sends can outrun a peer's clears.
            with tc.tile_critical(sync_engine=mybir.EngineType.Pool, name="clr"):
                for s in (*data_sem, *lsem, flush_sem):
                    nc.gpsimd.sem_clear(s)
            bar_i = dram.tile([1, 4], mybir.dt.int32)
            bar_o = dram.tile([C, 4], mybir.dt.int32)
            nc.gpsimd.collective_compute(
                "AllGather", mybir.AluOpType.bypass,
                replica_groups=[list(range(C))],
                ins=[bar_i[:, :]], outs=[bar_o[:, :]])

            def exchange(k):
                """Round k: push my region-0 shard of tb[k%2] to every peer's
                region (me ^ r), then flush the full table to DRAM once all 7
                remote shards have landed (14 = 7 dests * 2 sem incs).  Also
                waits for round k-1's sends so layer k's staging of round k+1
                can safely overwrite tb[(k+1)%2] region 0."""
                p = k % 2
                for r in range(1, C):
                    rdests = [None] * C
                    rdests[r] = (0, r)
                    nc.gpsimd.remote_dma_broadcast(
                        out_ap=tb[p][:, r * RD:(r + 1) * RD],
                        in_ap=tb[p][:, 0:RD],
                        remote_sem=data_sem[p], local_sem=lsem[p],
                        rdests=rdests, queue_num=0)
                nc.gpsimd.trigger_dma(count=None, queue_num=0)
                with tc.tile_critical(sync_engine=mybir.EngineType.SP,
                                      name=f"xchg{k}"):
                    nc.sync.wait_ge(data_sem[p], 14 * (k // 2 + 1))
                    if 1 <= k <= 5:
                        nc.sync.wait_ge(lsem[(k + 1) % 2],
                                        7 * 16 * ((k + 1) // 2))
                    nc.sync.dma_start(
                        table_d[p].rearrange("(q w) d -> q (w d)", q=128),
                        tb[p][:]).then_inc(flush_sem, 16)
                    nc.sync.wait_ge(flush_sem, 16 * (k + 1))

            # ---- phase A: xw0 = X @ W0 ----
            for t in range(NT):
                R = min(128, NPC - t * 128)
                xt = work.tile([128, IN_DIM], mybir.dt.float32, name="xt", tag="xt")
                if R < 128:
                    nc.vector.memset(xt[:], 0.0)
                nc.sync.dma_start(xt[0:R, :], x_in[t * 128:t * 128 + R, :])
                psum_xw = pxw.tile([128, D], mybir.dt.float32, name="pxw_a", tag="pxw")
                for k in range(KCH):
                    ptr = pbig.tile([128, 128], mybir.dt.float32, name="ptr", tag="pbig")
                    nc.tensor.transpose(out=ptr[:], in_=xt[:, k * 128:(k + 1) * 128],
                                        identity=ident[:])
                    xtT = work.tile([128, 128], mybir.dt.float32, name="xtT", tag="xtT")
                    nc.vector.tensor_copy(out=xtT[:], in_=ptr[:])
                    nc.tensor.matmul(out=psum_xw[:], lhsT=xtT[:],
                                     rhs=w0_sb[:, k * D:(k + 1) * D],
                                     start=(k == 0), stop=(k == KCH - 1))
                nc.scalar.activation(out=tb[0][:, t * D:(t + 1) * D],
                                     in_=psum_xw[:],
                                     func=mybir.ActivationFunctionType.Copy)
            exchange(0)

            # ---- layers ----
            ht_cur, ht_nxt = ht_a, ht_b
            for l in range(7):
                lines = table_d[l % 2].rearrange("(l f) d -> l (f d)", f=4)
                for t in range(NT):
                    g = gp.tile([128, NB * D], mybir.dt.float16, name=f"g{l}_{t}", tag="g")
                    for q, (r, b0, nb) in enumerate(splits):
                        ni = nb * 128
                        dma_gather_raw(
                            nc.gpsimd,
                            out_ap=g[:, b0 * D:(b0 + nb) * D].rearrange(
                                "p (m d) -> p m d", d=D),
                            in_ap=lines[:, r * D:(r + 1) * D],
                            idxs_ap=idx_sb[:, (t * NSLOT + b0 * 128) // 16:
                                              (t * NSLOT + (b0 + nb) * 128) // 16],
                            num_idxs=ni, elem_size=D, elem_step=128,
                            queue_num=q % 4)
                    psum = psc.tile([128, 128], mybir.dt.float32, name="psc", tag="psc")
                    ngrp = [0, 0, 0, 0]
                    for j in range(NB):
                        ngrp[j % 4] += 1
                    cnt = [0, 0, 0, 0]
                    for j in range(NB):
                        cg = j % 4
                        cnt[cg] += 1
                        nc.tensor.matmul(
                            out=psum[32 * cg:32 * cg + 32, :],
                            lhsT=g[:, j * D:(j + 1) * D],
                            rhs=s_sb[:, (t * NB + j) * 128:(t * NB + j + 1) * 128],
                            start=(cnt[cg] == 1), stop=(cnt[cg] == ngrp[cg]),
                            tile_position=(0, 32 * cg), skip_group_check=True)
                    red = work.tile([32, 128], mybir.dt.float32, name="red", tag="red")
                    nc.vector.tensor_copy(out=red[:], in_=psum[0:32, :])
                    for cg in range(1, 4):
                        if ngrp[cg]:
                            nc.vector.tensor_add(out=red[:], in0=red[:],
                                                 in1=psum[32 * cg:32 * cg + 32, :])
                    if l < 6:
                        nc.scalar.activation(
                            out=ht_nxt[:, t * 128:(t + 1) * 128], in_=red[:],
                            func=mybir.ActivationFunctionType.Relu)
                        psum_xw = pxw.tile([128, D], mybir.dt.float32,
                                           name="pxw_l", tag="pxw")
                        nc.tensor.matmul(
                            out=psum_xw[:],
                            lhsT=ht_nxt[:, t * 128:(t + 1) * 128],
                            rhs=wl_sb[:, l * D:(l + 1) * D],
                            start=True, stop=True)
                        nc.scalar.activation(out=tb[(l + 1) % 2][:, t * D:(t + 1) * D],
                                             in_=psum_xw[:],
                                             func=mybir.ActivationFunctionType.Copy)
                    else:
                        nc.scalar.activation(
                            out=zt_sb[0:16, t * 128:(t + 1) * 128], in_=red[0:16, :],
                            func=mybir.ActivationFunctionType.Copy)
                if l < 6:
                    exchange(l + 1)
                ht_cur, ht_nxt = ht_nxt, ht_cur

            # ---- z AllGather (feature-major) ----
            nc.sync.dma_start(z_shard[:, :], zt_sb[0:16, 0:NPC])
            nc.gpsimd.collective_compute(
                "AllGather", mybir.AluOpType.bypass,
                replica_groups=[list(range(C))],
                ins=[z_shard[:, :]], outs=[z_stack[:, :]])
            for d in range(C):
                nc.sync.dma_start(z_all[0:16, d * NPC:(d + 1) * NPC],
                                  z_stack[d * 16:(d + 1) * 16, :])

            # ---- decoder (fp16 output, scaled by out_scale) ----
            if VARIANT != "nodec":
                for t in range(NT):
                    R = min(128, NPC - t * 128)
                    for cg in range(0, NCH, DST):
                        ncc = min(DST, NCH - cg)
                        st = dstage.tile([128, DST * CHW], mybir.dt.float16,
                                         name="dst", tag="dst")
                        for i in range(ncc):
                            ch = cg + i
                            pd = pbig.tile([128, CHW], mybir.dt.float32,
                                           name="pd", tag="pbig")
                            nc.tensor.matmul(
                                out=pd[:],
                                lhsT=zt_sb[0:16, t * 128:(t + 1) * 128],
                                rhs=z_all[0:16, ch * CHW:(ch + 1) * CHW],
                                start=True, stop=True)
                            if i % 2 == 0:
                                nc.vector.tensor_scalar_mul(
                                    out=st[:, i * CHW:(i + 1) * CHW], in0=pd[:],
                                    scalar1=float(out_scale))
                            else:
                                nc.scalar.activation(
                                    out=st[:, i * CHW:(i + 1) * CHW], in_=pd[:],
                                    func=mybir.ActivationFunctionType.Copy,
                                    scale=float(out_scale))
                        nc.sync.dma_start(
                            out_d[t * 128:t * 128 + R, cg * CHW:(cg + ncc) * CHW],
                            st[0:R, 0:ncc * CHW])
    nc.compile()
    return nc


_CACHE = {}


def _get_program(N, IN_DIM, NPC, NT, NBr, out_scale):
    key = (N, IN_DIM, NPC, NT, tuple(NBr), float(out_scale))
    if key not in _CACHE:
        _CACHE[key] = _build(N, IN_DIM, NPC, NT, NBr, out_scale)
    return _CACHE[key]


def _calibrate_scales(features, adj_row, adj_col, adj_vals, Ws):
    """Cheap fp32 host forward pass to pick per-layer normalizers alpha_l so
    the fp16 activation tables stay near max-abs 64 (relu is positively
    homogeneous, so scaling commutes; undone once on the host).  Also returns
    the max row 2-norm of the scaled z, bounding decoder products."""
    N = features.shape[0]
    rows = np.asarray(adj_row).astype(np.int64)
    cols = np.asarray(adj_col).astype(np.int64)
    vals = np.asarray(adj_vals).astype(np.float32)

    def spmm(x):
        out = np.zeros((N, x.shape[1]), np.float32)
        np.add.at(out, rows, vals[:, None] * x[cols])
        return out

    alphas = []
    h = np.asarray(features, np.float32)
    for l in range(7):
        u = h @ np.asarray(Ws[l], np.float32)
        m = float(np.abs(u).max()) or 1.0
        alphas.append(64.0 / m)
        a = spmm(u)
        h = np.maximum(a, 0) if l < 6 else a
    z_scaled = h * alphas[6]
    znorm = float(np.sqrt((z_scaled.astype(np.float64) ** 2).sum(1)).max()) or 1.0
    return alphas, znorm


def _make_in_maps(features, adj_row, adj_col, adj_vals, Ws):
    N, IN_DIM = features.shape
    NPC = N // C
    NT = (NPC + 127) // 128
    reg_of = _get_reg_of()
    per_core, NBr = _preprocess(adj_row, adj_col, adj_vals, N, NPC, NT, reg_of)
    alphas, znorm = _calibrate_scales(features, adj_row, adj_col, adj_vals, Ws)

    D = 32
    KCH = IN_DIM // 128
    # device weight l is W_l * alpha_l / alpha_{l-1} (alpha_{-1}=1); the
    # decoder output is scaled by out_scale so the z_s @ z_s^T products
    # (bounded by znorm^2) stay in fp16 range; the host undoes
    # out_scale * alpha_6^2 once.
    W0 = np.asarray(Ws[0], np.float32) * alphas[0]
    w0_host = W0.reshape(KCH, 128, D).transpose(1, 0, 2).reshape(128, KCH * D)
    wl_host = np.zeros((6, D, D), np.float32)
    for i in range(1, 7):
        w = np.asarray(Ws[i], np.float32) * (alphas[i] / alphas[i - 1])
        wl_host[i - 1, :, :w.shape[1]] = w
    wl_host = wl_host.reshape(6 * D, D)
    out_scale = 30000.0 / (znorm * znorm)
    unscale = 1.0 / (out_scale * alphas[6] ** 2)

    feats = np.asarray(features, np.float32)
    in_maps = []
    for c in range(C):
        s_host, idx_core = per_core[c]
        in_maps.append({
            "x_in": np.ascontiguousarray(feats[c * NPC:(c + 1) * NPC]),
            "s_in": s_host,
            "idx_in": idx_core,
            "w0_in": w0_host,
            "wl_in": wl_host,
        })
    return in_maps, (N, IN_DIM, NPC, NT, tuple(NBr), out_scale), unscale


def _postprocess(out_stack, dims, unscale):
    """out_stack: [C, NPC, N] fp16 device output -> full [N, N] fp32."""
    N = dims[0]
    return np.asarray(out_stack, np.float32).reshape(N, N) * np.float32(unscale)


def kernel(features, adj_row, adj_col, adj_vals, W0, W1, W2, W3, W4, W5, W6):
    in_maps, dims, unscale = _make_in_maps(features, adj_row, adj_col, adj_vals,
                                           [W0, W1, W2, W3, W4, W5, W6])
    nc = _get_program(*dims)
    res = bass_utils.run_bass_kernel_spmd(nc, in_maps, core_ids=list(range(C)))
    full = _postprocess(np.stack([res.results[c]["out"] for c in range(C)]),
                        dims, unscale)
    return full.reshape(-1)
